# revision 1
# baseline (speedup 1.0000x reference)
"""GPT forward kernel for 8 TRN2 NeuronCores.

Data-parallel over batch (B=8 -> 1 sequence per core). Host pre-transposes
weights to put the contraction dim on SBUF partitions, casts them to bf16,
and performs the (tiny) embedding gather. The device kernel keeps the
residual stream transposed (x^T [D, T], fp32 in SBUF) and runs the 8
transformer layers plus the vocab head entirely out of SBUF/PSUM.
"""
import sys
sys.path.insert(0, '/opt/trn_rl_repo')
import numpy as np
import ml_dtypes

import concourse.bass as bass
import concourse.tile as tile
from concourse import bacc, mybir
from concourse.bass_utils import run_bass_kernel_spmd

B, T, D, H, L, V, MAXT = 8, 1024, 1024, 16, 8, 8192, 4096
HD = D // H          # 64
P = 128
DS = D // P          # 8 d-subtiles
TS = T // P          # 8 t-subtiles
D2S = (2 * D) // P   # 16 mlp subtiles
VS = V // 512        # 16 vocab chunks
NCH = 512
EPS = 1e-5
SCALE = 1.0 / np.sqrt(HD)

F32 = mybir.dt.float32
BF16 = mybir.dt.bfloat16
AF = mybir.ActivationFunctionType
ALU = mybir.AluOpType

# smalls[:, col] layout, per layer base = l*80
SM_BQ, SM_BK, SM_BO, SM_B2, SM_B1 = 0, 8, 16, 24, 32
SM_L1W, SM_L1B, SM_L2W, SM_L2B = 48, 56, 64, 72
SM_PER_LAYER = 80
SM_LNFW, SM_LNFB = L * 80, L * 80 + 8
SM_COLS = L * 80 + 16

TRACE = False
LAST_RESULTS = None


def _layernorm(nc, pps, ppb, px, pr, xT, dst, sm, wcol, bcol, ones_cb, ones_rf, eps_t):
    """dst (bf16 [P, DS, T]) = LN(xT) * w + b; stats over partitions via PE.

    Stats matmuls run in bf16 (4x the fp32 PE rate); the per-element rounding
    is random so it averages out over the 1024-partition reduction."""
    for c in range(2):
        tch = bass.ts(c, NCH)
        mv = pps.tile([1, NCH], F32, tag="a")
        for k in range(DS):
            xb = px.tile([P, NCH], BF16, tag="xb")
            nc.vector.tensor_copy(xb[:], xT[:, k, tch])
            nc.tensor.matmul(mv[:], ones_cb[:], xb[:],
                             start=(k == 0), stop=(k == DS - 1))
        sq = pps.tile([1, NCH], F32, tag="a")
        for k in range(DS):
            sqt = px.tile([P, NCH], BF16, tag="sqb")
            nc.scalar.activation(sqt[:], xT[:, k, tch], AF.Square)
            nc.tensor.matmul(sq[:], ones_cb[:], sqt[:],
                             start=(k == 0), stop=(k == DS - 1))
        mrow = pr.tile([1, NCH], F32, tag="r")
        nc.vector.tensor_scalar_mul(mrow[:], mv[:], 1.0 / D)
        ex2 = pr.tile([1, NCH], F32, tag="r")
        nc.vector.tensor_scalar_mul(ex2[:], sq[:], 1.0 / D)
        m2 = pr.tile([1, NCH], F32, tag="r")
        nc.vector.tensor_mul(m2[:], mrow[:], mrow[:])
        var = pr.tile([1, NCH], F32, tag="r")
        nc.vector.tensor_sub(var[:], ex2[:], m2[:])
        sd = pr.tile([1, NCH], F32, tag="r")
        nc.scalar.activation(sd[:], var[:], AF.Sqrt, bias=eps_t[:])
        srow = pr.tile([1, NCH], F32, tag="r")
        nc.vector.reciprocal(srow[:], sd[:])
        nm = pr.tile([1, NCH], F32, tag="r")
        nc.vector.scalar_tensor_tensor(nm[:], mrow[:], -1.0, srow[:],
                                       op0=ALU.mult, op1=ALU.mult)
        A = ppb.tile([P, NCH], F32, tag="b")
        nc.tensor.matmul(A[:], ones_rf[:], srow[:], start=True, stop=True)
        Bb = ppb.tile([P, NCH], F32, tag="b")
        nc.tensor.matmul(Bb[:], ones_rf[:], nm[:], start=True, stop=True)
        for k in range(DS):
            u = px.tile([P, NCH], F32, tag="sq")
            nc.vector.tensor_mul(u[:], xT[:, k, tch], A[:])
            nc.vector.tensor_add(u[:], u[:], Bb[:])
            nc.vector.scalar_tensor_tensor(
                dst[:, k, tch], u[:], sm[:, wcol + k:wcol + k + 1],
                sm[:, bcol + k:bcol + k + 1].to_broadcast((P, NCH)),
                op0=ALU.mult, op1=ALU.add)


def _build(repeat=1):
    import contextlib
    nc = bacc.Bacc("TRN2", target_bir_lowering=False)

    x0T_d = nc.dram_tensor("x0T", [D, T], F32, kind="ExternalInput")
    WqT_d = nc.dram_tensor("WqT", [L, D, D], BF16, kind="ExternalInput")
    WkT_d = nc.dram_tensor("WkT", [L, D, D], BF16, kind="ExternalInput")
    WvT_d = nc.dram_tensor("WvT", [L, D, D], BF16, kind="ExternalInput")
    WoT_d = nc.dram_tensor("WoT", [L, D, D], BF16, kind="ExternalInput")
    W1T_d = nc.dram_tensor("W1T", [L, D, 2 * D], BF16, kind="ExternalInput")
    W2T_d = nc.dram_tensor("W2T", [L, 2 * D, D], BF16, kind="ExternalInput")
    hT_w_d = nc.dram_tensor("headT", [D, V], BF16, kind="ExternalInput")
    sm_d = nc.dram_tensor("smalls", [P, SM_COLS], F32, kind="ExternalInput")
    bv_d = nc.dram_tensor("bvB", [1, L * D], BF16, kind="ExternalInput")
    mask_d = nc.dram_tensor("mask01", [P, P], BF16, kind="ExternalInput")
    out_d = nc.dram_tensor("logits", [T, V], F32, kind="ExternalOutput")

    out_r = out_d[:, :].rearrange("(t pi) v -> pi t v", pi=P)

    with tile.TileContext(nc) as tc:
        with (
            tc.tile_pool(name="pc", bufs=1) as pc,
            tc.tile_pool(name="pw", bufs=3) as pw,
            tc.tile_pool(name="pw2", bufs=2) as pw2,
            tc.tile_pool(name="pbv", bufs=2) as pbv,
            tc.tile_pool(name="ppt", bufs=2) as ppt,
            tc.tile_pool(name="px", bufs=2) as px,
            tc.tile_pool(name="pr", bufs=5) as pr,
            tc.tile_pool(name="pps", bufs=4, space="PSUM") as pps,
            tc.tile_pool(name="ppo", bufs=2, space="PSUM") as ppo,
            tc.tile_pool(name="ppb", bufs=2, space="PSUM") as ppb,
        ):
            xT = pc.tile([P, DS, T], F32)
            hT = pc.tile([P, DS, T], BF16)
            qkT = pc.tile([P, 2 * DS, T], BF16)   # q rows 0:8, k rows 8:16; reused as gT
            Vg = pc.tile([P, TS, H, HD + 1], BF16)
            yT = hT  # attention output reuses the LN buffer (phases are disjoint)
            sm = pc.tile([P, SM_COLS], F32)
            mask = pc.tile([P, P], BF16)
            ones_r = pc.tile([1, P], BF16)
            ones_rf = pc.tile([1, P], F32)
            ones_cb = pc.tile([P, 1], BF16)
            eps_t = pc.tile([1, 1], F32)

            nc.vector.memset(ones_r[:], 1.0)
            nc.vector.memset(ones_rf[:], 1.0)
            nc.vector.memset(ones_cb[:], 1.0)
            nc.vector.memset(eps_t[:], EPS)
            nc.vector.memset(Vg[:, :, :, HD:HD + 1], 1.0)
            nc.sync.dma_start(sm[:], sm_d[:, :])
            nc.sync.dma_start(mask[:], mask_d[:, :])
            loop_cm = tc.For_i(0, repeat, 1) if repeat > 1 else contextlib.nullcontext()
            with loop_cm:
                nc.sync.dma_start(xT[:], x0T_d[:, :].rearrange("(po pi) t -> pi po t", pi=P))

                for l in range(L):
                    base = l * SM_PER_LAYER
                    _layernorm(nc, pps, ppb, px, pr, xT, hT, sm,
                               base + SM_L1W, base + SM_L1B, ones_cb, ones_rf, eps_t)

                    # ---- q^T / k^T projections: out[o, t] ----
                    for which, W_d, boff, qoff in ((0, WqT_d, SM_BQ, 0), (1, WkT_d, SM_BK, DS)):
                        for half in range(2):
                            wsl = pw.tile([P, DS, NCH], BF16, tag="w")
                            nc.sync.dma_start(
                                wsl[:],
                                W_d[l].rearrange("(po pi) o -> pi po o", pi=P)[:, :, bass.ts(half, NCH)])
                            for m in range(4):
                                mo = half * 4 + m
                                ps0 = pps.tile([P, NCH], F32, tag="a")
                                ps1 = pps.tile([P, NCH], F32, tag="a")
                                for k in range(DS):
                                    nc.tensor.matmul(ps0[:], wsl[:, k, bass.ts(m, P)],
                                                     hT[:, k, bass.ts(0, NCH)],
                                                     start=(k == 0), stop=(k == DS - 1))
                                    nc.tensor.matmul(ps1[:], wsl[:, k, bass.ts(m, P)],
                                                     hT[:, k, bass.ts(1, NCH)],
                                                     start=(k == 0), stop=(k == DS - 1))
                                for c, psx in ((0, ps0), (1, ps1)):
                                    nc.vector.tensor_scalar_add(
                                        qkT[:, qoff + mo, bass.ts(c, NCH)], psx[:],
                                        sm[:, base + boff + mo:base + boff + mo + 1])

                    # ---- V projection: out[t, o] (natural) into Vg ----
                    for half in range(2):
                        wsl = pw.tile([P, DS, NCH], BF16, tag="w")
                        nc.sync.dma_start(
                            wsl[:],
                            WvT_d[l].rearrange("(po pi) o -> pi po o", pi=P)[:, :, bass.ts(half, NCH)])
                        bvs = pbv.tile([1, NCH], BF16, tag="bvs")
                        nc.sync.dma_start(bvs[:], bv_d[:, l * D + half * NCH:l * D + (half + 1) * NCH])
                        for t_ in range(TS):
                            ps_t = pps.tile([P, NCH], F32, tag="a")
                            for k in range(DS):
                                nc.tensor.matmul(ps_t[:], hT[:, k, bass.ts(t_, P)],
                                                 wsl[:, k, :],
                                                 start=(k == 0), stop=False)
                            nc.tensor.matmul(ps_t[:], ones_r[:], bvs[:],
                                             start=False, stop=True)
                            nc.vector.tensor_copy(
                                Vg[:, t_, 8 * half:8 * half + 8, 0:HD],
                                ps_t[:].rearrange("p (h d) -> p h d", d=HD))

                    # ---- attention, per head ----
                    for h in range(H):
                        pbase = (h % 2) * HD
                        sub = h // 2
                        for c in range(2):
                            PT = ppt.tile([P, TS, NCH], BF16, tag="pt")
                            ntk = 4 * c + 4
                            for tk in range(ntk):
                                ls = max(0, tk * P - c * NCH)
                                w_ = NCH - ls
                                sT = pps.tile([P, NCH], F32, tag="a")
                                nc.tensor.matmul(
                                    sT[:, :w_],
                                    qkT[pbase:pbase + HD, DS + sub, bass.ts(tk, P)],
                                    qkT[pbase:pbase + HD, sub, c * NCH + ls:(c + 1) * NCH],
                                    start=True, stop=True)
                                nc.scalar.activation(PT[:, tk, ls:], sT[:, :w_], AF.Exp,
                                                     scale=float(SCALE))
                                if tk >= 4 * c:
                                    nc.vector.tensor_mul(PT[:, tk, ls:ls + P],
                                                         PT[:, tk, ls:ls + P], mask[:])
                            po_t = ppo.tile([HD + 1, NCH], F32, tag="o")
                            for tk in range(ntk):
                                ls = max(0, tk * P - c * NCH)
                                nc.tensor.matmul(po_t[:, ls:], Vg[:, tk, h, :],
                                                 PT[:, tk, ls:],
                                                 start=(tk == 0), stop=(tk == ntk - 1))
                            dn = pr.tile([1, NCH], F32, tag="r")
                            nc.vector.reciprocal(dn[:], po_t[HD:HD + 1, :])
                            bc = ppb.tile([P, NCH], F32, tag="b")
                            nc.tensor.matmul(bc[:HD, :], ones_rf[:, :HD], dn[:],
                                             start=True, stop=True)
                            yu = px.tile([HD, NCH], BF16, tag="yu")
                            nc.vector.tensor_copy(yu[:], po_t[:HD, :])
                            nc.vector.tensor_mul(yT[pbase:pbase + HD, sub, bass.ts(c, NCH)],
                                                 yu[:], bc[:HD, :])

                    # ---- attention out projection + residual ----
                    for half in range(2):
                        wsl = pw.tile([P, DS, NCH], BF16, tag="w")
                        nc.sync.dma_start(
                            wsl[:],
                            WoT_d[l].rearrange("(po pi) o -> pi po o", pi=P)[:, :, bass.ts(half, NCH)])
                        for m in range(4):
                            mo = half * 4 + m
                            ps0 = pps.tile([P, NCH], F32, tag="a")
                            ps1 = pps.tile([P, NCH], F32, tag="a")
                            for k in range(DS):
                                nc.tensor.matmul(ps0[:], wsl[:, k, bass.ts(m, P)],
                                                 yT[:, k, bass.ts(0, NCH)],
                                                 start=(k == 0), stop=(k == DS - 1))
                                nc.tensor.matmul(ps1[:], wsl[:, k, bass.ts(m, P)],
                                                 yT[:, k, bass.ts(1, NCH)],
                                                 start=(k == 0), stop=(k == DS - 1))
                            for c, psx in ((0, ps0), (1, ps1)):
                                tch = bass.ts(c, NCH)
                                nc.vector.scalar_tensor_tensor(
                                    xT[:, mo, tch], psx[:],
                                    sm[:, base + SM_BO + mo:base + SM_BO + mo + 1],
                                    xT[:, mo, tch], op0=ALU.add, op1=ALU.add)

                    # ---- MLP ----
                    _layernorm(nc, pps, ppb, px, pr, xT, hT, sm,
                               base + SM_L2W, base + SM_L2B, ones_cb, ones_rf, eps_t)
                    for quarter in range(4):
                        wsl = pw.tile([P, DS, NCH], BF16, tag="w")
                        nc.sync.dma_start(
                            wsl[:],
                            W1T_d[l].rearrange("(po pi) o -> pi po o", pi=P)[:, :, bass.ts(quarter, NCH)])
                        for m in range(4):
                            mo = quarter * 4 + m
                            ps0 = pps.tile([P, NCH], F32, tag="a")
                            ps1 = pps.tile([P, NCH], F32, tag="a")
                            for k in range(DS):
                                nc.tensor.matmul(ps0[:], wsl[:, k, bass.ts(m, P)],
                                                 hT[:, k, bass.ts(0, NCH)],
                                                 start=(k == 0), stop=(k == DS - 1))
                                nc.tensor.matmul(ps1[:], wsl[:, k, bass.ts(m, P)],
                                                 hT[:, k, bass.ts(1, NCH)],
                                                 start=(k == 0), stop=(k == DS - 1))
                            for c, psx in ((0, ps0), (1, ps1)):
                                nc.scalar.activation(
                                    qkT[:, mo, bass.ts(c, NCH)], psx[:], AF.Gelu,
                                    bias=sm[:, base + SM_B1 + mo:base + SM_B1 + mo + 1])

                    for half in range(2):
                        w2t = pw2.tile([P, D2S, NCH], BF16, tag="w2")
                        nc.sync.dma_start(
                            w2t[:],
                            W2T_d[l].rearrange("(po pi) o -> pi po o", pi=P)[:, :, bass.ts(half, NCH)])
                        for m in range(4):
                            mo = half * 4 + m
                            ps0 = pps.tile([P, NCH], F32, tag="a")
                            ps1 = pps.tile([P, NCH], F32, tag="a")
                            for k in range(D2S):
                                nc.tensor.matmul(ps0[:], w2t[:, k, bass.ts(m, P)],
                                                 qkT[:, k, bass.ts(0, NCH)],
                                                 start=(k == 0), stop=(k == D2S - 1))
                                nc.tensor.matmul(ps1[:], w2t[:, k, bass.ts(m, P)],
                                                 qkT[:, k, bass.ts(1, NCH)],
                                                 start=(k == 0), stop=(k == D2S - 1))
                            for c, psx in ((0, ps0), (1, ps1)):
                                tch = bass.ts(c, NCH)
                                nc.vector.scalar_tensor_tensor(
                                    xT[:, mo, tch], psx[:],
                                    sm[:, base + SM_B2 + mo:base + SM_B2 + mo + 1],
                                    xT[:, mo, tch], op0=ALU.add, op1=ALU.add)

                # ---- final LN + vocab head ----
                _layernorm(nc, pps, ppb, px, pr, xT, hT, sm,
                           SM_LNFW, SM_LNFB, ones_cb, ones_rf, eps_t)
                hw_r = hT_w_d[:, :].rearrange("(po pi) v -> pi po v", pi=P)
                for vp in range(VS // 2):
                    ws0 = pw.tile([P, DS, NCH], BF16, tag="w")
                    nc.sync.dma_start(ws0[:], hw_r[:, :, bass.ts(2 * vp, NCH)])
                    ws1 = pw.tile([P, DS, NCH], BF16, tag="w")
                    nc.sync.dma_start(ws1[:], hw_r[:, :, bass.ts(2 * vp + 1, NCH)])
                    for t_ in range(TS):
                        ps0 = pps.tile([P, NCH], F32, tag="a")
                        ps1 = pps.tile([P, NCH], F32, tag="a")
                        for k in range(DS):
                            nc.tensor.matmul(ps0[:], hT[:, k, bass.ts(t_, P)],
                                             ws0[:, k, :],
                                             start=(k == 0), stop=(k == DS - 1))
                            nc.tensor.matmul(ps1[:], hT[:, k, bass.ts(t_, P)],
                                             ws1[:, k, :],
                                             start=(k == 0), stop=(k == DS - 1))
                        for j, psx in ((0, ps0), (1, ps1)):
                            ot = px.tile([P, NCH], F32, tag="ot")
                            nc.vector.tensor_copy(ot[:], psx[:])
                            nc.sync.dma_start(out_r[:, t_, bass.ts(2 * vp + j, NCH)], ot[:])

    nc.compile()
    return nc


_NC = {}


def _get_nc(repeat=1):
    if repeat not in _NC:
        _NC[repeat] = _build(repeat)
    return _NC[repeat]


def _pack_cols(vec, ncols):
    """[ncols*128] -> [128, ncols] with column j = vec[j*128:(j+1)*128]."""
    return np.ascontiguousarray(vec.reshape(ncols, P).T)


def kernel(idx, timesteps, tok_emb_w, pos_emb, global_pos_emb,
           ln1_w, ln1_b, Wq, bq, Wk, bk, Wv, bv, Wo, bo,
           ln2_w, ln2_b, W1, b1, W2, b2, lnf_w, lnf_b, head_w):
    global LAST_RESULTS
    f = lambda a: np.asarray(a, dtype=np.float32)
    idx = np.asarray(idx, dtype=np.int64)
    tsteps = np.asarray(timesteps, dtype=np.int64)
    tok_emb_w, pos_emb, global_pos_emb = f(tok_emb_w), f(pos_emb), f(global_pos_emb)

    # embedding on host (tiny compute, avoids on-device gather)
    x0 = tok_emb_w[idx] + global_pos_emb[0][tsteps[:, 0]][:, None, :] + pos_emb[:, :T]
    x0 = np.ascontiguousarray(x0.astype(np.float32))

    bf = lambda a: np.ascontiguousarray(np.asarray(a, np.float32)).astype(ml_dtypes.bfloat16)
    shared = {
        "WqT": bf(f(Wq).transpose(0, 2, 1)),
        "WkT": bf(f(Wk).transpose(0, 2, 1)),
        "WvT": bf(f(Wv).transpose(0, 2, 1)),
        "WoT": bf(f(Wo).transpose(0, 2, 1)),
        "W1T": bf(f(W1).transpose(0, 2, 1)),
        "W2T": bf(f(W2).transpose(0, 2, 1)),
        "headT": bf(f(head_w).T),
        "bvB": bf(f(bv).reshape(1, L * D)),
    }
    smalls = np.zeros((P, SM_COLS), np.float32)
    for l in range(L):
        b = l * SM_PER_LAYER
        smalls[:, b + SM_BQ:b + SM_BQ + 8] = _pack_cols(f(bq)[l], DS)
        smalls[:, b + SM_BK:b + SM_BK + 8] = _pack_cols(f(bk)[l], DS)
        smalls[:, b + SM_BO:b + SM_BO + 8] = _pack_cols(f(bo)[l], DS)
        smalls[:, b + SM_B2:b + SM_B2 + 8] = _pack_cols(f(b2)[l], DS)
        smalls[:, b + SM_B1:b + SM_B1 + 16] = _pack_cols(f(b1)[l], D2S)
        smalls[:, b + SM_L1W:b + SM_L1W + 8] = _pack_cols(f(ln1_w)[l], DS)
        smalls[:, b + SM_L1B:b + SM_L1B + 8] = _pack_cols(f(ln1_b)[l], DS)
        smalls[:, b + SM_L2W:b + SM_L2W + 8] = _pack_cols(f(ln2_w)[l], DS)
        smalls[:, b + SM_L2B:b + SM_L2B + 8] = _pack_cols(f(ln2_b)[l], DS)
    smalls[:, SM_LNFW:SM_LNFW + 8] = _pack_cols(f(lnf_w), DS)
    smalls[:, SM_LNFB:SM_LNFB + 8] = _pack_cols(f(lnf_b), DS)
    shared["smalls"] = smalls

    m01 = (np.arange(P)[:, None] <= np.arange(P)[None, :])
    shared["mask01"] = m01.astype(ml_dtypes.bfloat16)

    in_maps = []
    for b_ in range(B):
        m = dict(shared)
        m["x0T"] = np.ascontiguousarray(x0[b_].T)
        in_maps.append(m)

    global LAST_IN_MAPS
    LAST_IN_MAPS = in_maps
    nc = _get_nc()
    res = run_bass_kernel_spmd(nc, in_maps, core_ids=list(range(B)), trace=TRACE)
    LAST_RESULTS = res
    out = np.stack([np.asarray(res.results[c]["logits"], np.float32) for c in range(B)])
    return out


# ---------------------------------------------------------------------------
# Timing helpers (test-only): replicate run_bass_via_pjrt's sharded jit with
# device-resident inputs so repeated calls measure (dispatch + NEFF exec).
# ---------------------------------------------------------------------------
LAST_IN_MAPS = None


def _sharded_exec(nc, in_maps):
    import jax
    from jax.experimental.shard_map import shard_map
    from jax.sharding import Mesh, PartitionSpec
    from concourse import bass2jax

    bass2jax.install_neuronx_cc_hook()
    n_cores = len(in_maps)
    partition_name = nc.partition_id_tensor.name if nc.partition_id_tensor else None
    in_names, out_names, out_avals, zero_outs = [], [], [], []
    for alloc in nc.m.functions[0].allocations:
        if not isinstance(alloc, mybir.MemoryLocationSet):
            continue
        name = alloc.memorylocations[0].name
        if alloc.kind == "ExternalInput":
            if name != partition_name:
                in_names.append(name)
        elif alloc.kind == "ExternalOutput":
            shape = tuple(alloc.tensor_shape)
            dtype = mybir.dt.np(alloc.dtype)
            out_names.append(name)
            out_avals.append(jax.core.ShapedArray(shape, dtype))
            zero_outs.append(np.zeros(shape, dtype))
    n_params = len(in_names)
    n_outs = len(out_avals)
    all_in_names = list(in_names) + list(out_names)
    if partition_name is not None:
        all_in_names.append(partition_name)
    donate = tuple(range(n_params, n_params + n_outs))

    def _body(*args):
        operands = list(args)
        if partition_name is not None:
            operands.append(bass2jax.partition_id_tensor())
        outs = bass2jax._bass_exec_p.bind(
            *operands,
            out_avals=tuple(out_avals),
            in_names=tuple(all_in_names),
            out_names=tuple(out_names),
            lowering_input_output_aliases=(),
            sim_require_finite=True,
            sim_require_nnan=True,
            nc=nc,
        )
        return tuple(outs)

    devices = jax.devices()[:n_cores]
    mesh = Mesh(np.asarray(devices), ("core",))
    sharded = jax.jit(
        shard_map(_body, mesh=mesh,
                  in_specs=(PartitionSpec("core"),) * (n_params + n_outs),
                  out_specs=(PartitionSpec("core"),) * n_outs,
                  check_rep=False),
        donate_argnums=donate, keep_unused=True)

    concat_in = [np.concatenate([np.asarray(m[name]) for m in in_maps], axis=0)
                 for name in in_names]
    concat_zeros = [np.zeros((n_cores * z.shape[0], *z.shape[1:]), z.dtype)
                    for z in zero_outs]
    from jax.sharding import NamedSharding
    sh = NamedSharding(mesh, PartitionSpec("core"))
    dev_in = [jax.device_put(a, sh) for a in concat_in]
    return sharded, dev_in, concat_zeros, sh


def _time_exec(nc, in_maps, iters):
    import time as _time
    import jax
    sharded, dev_in, concat_zeros, sh = _sharded_exec(nc, in_maps)
    times = []
    for _ in range(iters):
        zs = [jax.device_put(z, sh) for z in concat_zeros]
        jax.block_until_ready(zs)
        jax.block_until_ready(dev_in)
        t0 = _time.perf_counter()
        out = sharded(*dev_in, *zs)
        jax.block_until_ready(out)
        times.append(_time.perf_counter() - t0)
    return times


def timed_run(iters=5):
    assert LAST_IN_MAPS is not None, "call kernel() first"
    return _time_exec(_get_nc(), LAST_IN_MAPS, iters)


_BASELINE_NC = None


def _build_baseline():
    nc = bacc.Bacc("TRN2", target_bir_lowering=False)
    a_d = nc.dram_tensor("a", [P, P], F32, kind="ExternalInput")
    o_d = nc.dram_tensor("o", [P, P], F32, kind="ExternalOutput")
    with tile.TileContext(nc) as tc:
        with tc.tile_pool(name="s", bufs=1) as s:
            t = s.tile([P, P], F32)
            nc.sync.dma_start(t[:], a_d[:, :])
            nc.sync.dma_start(o_d[:, :], t[:])
    nc.compile()
    return nc


def timed_baseline(iters=5):
    global _BASELINE_NC
    if _BASELINE_NC is None:
        _BASELINE_NC = _build_baseline()
    maps = [{"a": np.zeros((P, P), np.float32)} for _ in range(B)]
    return _time_exec(_BASELINE_NC, maps, iters)


def timed_slope(ns=(1, 4, 12), zsets=12):
    """Async-dispatch n calls back-to-back; slope of total-time vs n ~ exec."""
    import time as _time
    import jax
    assert LAST_IN_MAPS is not None
    sharded, dev_in, concat_zeros, sh = _sharded_exec(_get_nc(), LAST_IN_MAPS)
    all_zs = [[jax.device_put(z, sh) for z in concat_zeros] for _ in range(zsets)]
    jax.block_until_ready(all_zs)
    jax.block_until_ready(dev_in)
    # warm
    out = sharded(*dev_in, *all_zs[0])
    jax.block_until_ready(out)
    res = {}
    for n in ns:
        zs_fresh = [[jax.device_put(z, sh) for z in concat_zeros] for _ in range(n)]
        jax.block_until_ready(zs_fresh)
        t0 = _time.perf_counter()
        outs = [sharded(*dev_in, *zs_fresh[i]) for i in range(n)]
        jax.block_until_ready(outs)
        res[n] = _time.perf_counter() - t0
    return res


def timed_slope_nc(nc, in_maps, ns=(4, 20)):
    import time as _time
    import jax
    sharded, dev_in, concat_zeros, sh = _sharded_exec(nc, in_maps)
    out = sharded(*dev_in, *[jax.device_put(z, sh) for z in concat_zeros])
    jax.block_until_ready(out)
    res = {}
    for n in ns:
        zs_fresh = [[jax.device_put(z, sh) for z in concat_zeros] for _ in range(n)]
        jax.block_until_ready(zs_fresh)
        jax.block_until_ready(dev_in)
        t0 = _time.perf_counter()
        outs = [sharded(*dev_in, *zs_fresh[i]) for i in range(n)]
        jax.block_until_ready(outs)
        res[n] = _time.perf_counter() - t0
    return res


def timed_repeat(r=5, iters=6):
    """exec_ns ~= (min_time(R=r NEFF) - min_time(R=1 NEFF)) / (r-1)."""
    assert LAST_IN_MAPS is not None
    t1 = min(_time_exec(_get_nc(1), LAST_IN_MAPS, iters))
    tr = min(_time_exec(_get_nc(r), LAST_IN_MAPS, iters))
    return (tr - t1) / (r - 1), t1, tr



# revision 2
# speedup vs baseline: 1.9783x; 1.9783x over previous
"""GPT forward kernel for 8 TRN2 NeuronCores.

Data-parallel over batch (B=8 -> 1 sequence per core). Host pre-transposes
weights to put the contraction dim on SBUF partitions, casts them to bf16,
and performs the (tiny) embedding gather. The device kernel keeps the
residual stream transposed (x^T [D, T], fp32 in SBUF) and runs the 8
transformer layers plus the vocab head entirely out of SBUF/PSUM.
"""
import sys
sys.path.insert(0, '/opt/trn_rl_repo')
import numpy as np
import ml_dtypes

import concourse.bass as bass
import concourse.tile as tile
from concourse import bacc, mybir
from concourse.bass_utils import run_bass_kernel_spmd

B, T, D, H, L, V, MAXT = 8, 1024, 1024, 16, 8, 8192, 4096
HD = D // H          # 64
P = 128
DS = D // P          # 8 d-subtiles
TS = T // P          # 8 t-subtiles
D2S = (2 * D) // P   # 16 mlp subtiles
VS = V // 512        # 16 vocab chunks
NCH = 512
EPS = 1e-5
SCALE = 1.0 / np.sqrt(HD)

F32 = mybir.dt.float32
BF16 = mybir.dt.bfloat16
AF = mybir.ActivationFunctionType
ALU = mybir.AluOpType

# smalls[:, col] layout, per layer base = l*80
SM_BQ, SM_BK, SM_BO, SM_B2, SM_B1 = 0, 8, 16, 24, 32
SM_L1W, SM_L1B, SM_L2W, SM_L2B = 48, 56, 64, 72
SM_PER_LAYER = 80
SM_LNFW, SM_LNFB = L * 80, L * 80 + 8
SM_COLS = L * 80 + 16

TRACE = False
LAST_RESULTS = None


def _layernorm(nc, pps, ppb, px, pr, xT, dst, sm, wcol, bcol, ones_cb, ones_rf, eps_t):
    """dst (bf16 [P, DS, T]) = (xT - mean) / std; stats over partitions via PE.

    gamma/beta are folded into the consuming projection weights host-side, so
    the tail is just x*A + B. Stats matmuls run in bf16 (4x the fp32 PE rate);
    the broadcast matmuls run in bf16 too (srow/nm quantized to bf16 — their
    consumers are bf16 anyway)."""
    for c in range(2):
        tch = bass.ts(c, NCH)
        mv = pps.tile([1, NCH], F32, tag="a")
        for k in range(DS):
            xb = px.tile([P, NCH], BF16, tag="xb")
            nc.vector.tensor_copy(xb[:], xT[:, k, tch])
            nc.tensor.matmul(mv[:], ones_cb[:], xb[:],
                             start=(k == 0), stop=(k == DS - 1))
        sq = pps.tile([1, NCH], F32, tag="a")
        for k in range(DS):
            sqt = px.tile([P, NCH], BF16, tag="sqb")
            nc.scalar.activation(sqt[:], xT[:, k, tch], AF.Square)
            nc.tensor.matmul(sq[:], ones_cb[:], sqt[:],
                             start=(k == 0), stop=(k == DS - 1))
        mrow = pr.tile([1, NCH], F32, tag="r")
        nc.vector.tensor_scalar_mul(mrow[:], mv[:], 1.0 / D)
        ex2 = pr.tile([1, NCH], F32, tag="r")
        nc.vector.tensor_scalar_mul(ex2[:], sq[:], 1.0 / D)
        m2 = pr.tile([1, NCH], F32, tag="r")
        nc.vector.tensor_mul(m2[:], mrow[:], mrow[:])
        var = pr.tile([1, NCH], F32, tag="r")
        nc.vector.tensor_sub(var[:], ex2[:], m2[:])
        sd = pr.tile([1, NCH], F32, tag="r")
        nc.scalar.activation(sd[:], var[:], AF.Sqrt, bias=eps_t[:])
        srow = pr.tile([1, NCH], BF16, tag="rb")
        with nc.allow_low_precision(reason="LN scale bf16; consumers are bf16"):
            nc.vector.reciprocal(srow[:], sd[:])
            nm = pr.tile([1, NCH], BF16, tag="rb")
            nc.vector.scalar_tensor_tensor(nm[:], mrow[:], -1.0, srow[:],
                                           op0=ALU.mult, op1=ALU.mult)
        A = ppb.tile([P, NCH], F32, tag="b")
        nc.tensor.matmul(A[:], ones_rf[:], srow[:], start=True, stop=True)
        Bb = ppb.tile([P, NCH], F32, tag="b")
        nc.tensor.matmul(Bb[:], ones_rf[:], nm[:], start=True, stop=True)
        for k in range(DS):
            u = px.tile([P, NCH], F32, tag="sq")
            nc.vector.tensor_mul(u[:], xT[:, k, tch], A[:])
            nc.vector.tensor_add(dst[:, k, tch], u[:], Bb[:])


def _build(repeat=1):
    import contextlib
    nc = bacc.Bacc("TRN2", target_bir_lowering=False)

    x0T_d = nc.dram_tensor("x0T", [D, T], F32, kind="ExternalInput")
    WqT_d = nc.dram_tensor("WqT", [L, D, D], BF16, kind="ExternalInput")
    WkT_d = nc.dram_tensor("WkT", [L, D, D], BF16, kind="ExternalInput")
    WvT_d = nc.dram_tensor("WvT", [L, D, D], BF16, kind="ExternalInput")
    WoT_d = nc.dram_tensor("WoT", [L, D, D], BF16, kind="ExternalInput")
    W1T_d = nc.dram_tensor("W1T", [L, D, 2 * D], BF16, kind="ExternalInput")
    W2T_d = nc.dram_tensor("W2T", [L, 2 * D, D], BF16, kind="ExternalInput")
    hT_w_d = nc.dram_tensor("headT", [D, V], BF16, kind="ExternalInput")
    sm_d = nc.dram_tensor("smalls", [P, SM_COLS], F32, kind="ExternalInput")
    bv_d = nc.dram_tensor("bvB", [1, L * D], BF16, kind="ExternalInput")
    mask_d = nc.dram_tensor("mask01", [P, P], BF16, kind="ExternalInput")
    out_d = nc.dram_tensor("logits", [T, V], F32, kind="ExternalOutput")

    out_r = out_d[:, :].rearrange("(t pi) v -> pi t v", pi=P)

    with tile.TileContext(nc) as tc:
        with (
            tc.tile_pool(name="pc", bufs=1) as pc,
            tc.tile_pool(name="pw", bufs=3) as pw,
            tc.tile_pool(name="pw2", bufs=2) as pw2,
            tc.tile_pool(name="pbv", bufs=2) as pbv,
            tc.tile_pool(name="ppt", bufs=2) as ppt,
            tc.tile_pool(name="px", bufs=2) as px,
            tc.tile_pool(name="pr", bufs=5) as pr,
            tc.tile_pool(name="pps", bufs=4, space="PSUM") as pps,
            tc.tile_pool(name="ppo", bufs=2, space="PSUM") as ppo,
            tc.tile_pool(name="ppb", bufs=2, space="PSUM") as ppb,
        ):
            xT = pc.tile([P, DS, T], F32)
            hT = pc.tile([P, DS, T], BF16)
            qkT = pc.tile([P, 2 * DS, T], BF16)   # q rows 0:8, k rows 8:16; reused as gT
            Vg = pc.tile([P, TS, H, HD + 1], BF16)
            yT = hT  # attention output reuses the LN buffer (phases are disjoint)
            sm = pc.tile([P, SM_COLS], F32)
            mask = pc.tile([P, P], BF16)
            ones_r = pc.tile([1, P], BF16)
            ones_rf = pc.tile([1, P], BF16)
            ones_cb = pc.tile([P, 1], BF16)
            eps_t = pc.tile([1, 1], F32)

            nc.vector.memset(ones_r[:], 1.0)
            nc.vector.memset(ones_rf[:], 1.0)
            nc.vector.memset(ones_cb[:], 1.0)
            nc.vector.memset(eps_t[:], EPS)
            nc.vector.memset(Vg[:, :, :, HD:HD + 1], 1.0)
            nc.sync.dma_start(sm[:], sm_d[:, :])
            nc.sync.dma_start(mask[:], mask_d[:, :])
            loop_cm = tc.For_i(0, repeat, 1) if repeat > 1 else contextlib.nullcontext()
            with loop_cm:
                nc.sync.dma_start(xT[:], x0T_d[:, :].rearrange("(po pi) t -> pi po t", pi=P))

                for l in range(L):
                    base = l * SM_PER_LAYER
                    _layernorm(nc, pps, ppb, px, pr, xT, hT, sm,
                               base + SM_L1W, base + SM_L1B, ones_cb, ones_rf, eps_t)

                    # ---- q^T / k^T projections: out[o, t] ----
                    for which, W_d, boff, qoff in ((0, WqT_d, SM_BQ, 0), (1, WkT_d, SM_BK, DS)):
                        for half in range(2):
                            wsl = pw.tile([P, DS, NCH], BF16, tag="w")
                            nc.sync.dma_start(
                                wsl[:],
                                W_d[l].rearrange("(po pi) o -> pi po o", pi=P)[:, :, bass.ts(half, NCH)])
                            for m in range(4):
                                mo = half * 4 + m
                                ps0 = pps.tile([P, NCH], F32, tag="a")
                                ps1 = pps.tile([P, NCH], F32, tag="a")
                                for k in range(DS):
                                    nc.tensor.matmul(ps0[:], wsl[:, k, bass.ts(m, P)],
                                                     hT[:, k, bass.ts(0, NCH)],
                                                     start=(k == 0), stop=(k == DS - 1))
                                    nc.tensor.matmul(ps1[:], wsl[:, k, bass.ts(m, P)],
                                                     hT[:, k, bass.ts(1, NCH)],
                                                     start=(k == 0), stop=(k == DS - 1))
                                for c, psx in ((0, ps0), (1, ps1)):
                                    nc.vector.tensor_scalar_add(
                                        qkT[:, qoff + mo, bass.ts(c, NCH)], psx[:],
                                        sm[:, base + boff + mo:base + boff + mo + 1])

                    # ---- V projection: out[t, o] (natural) into Vg ----
                    for half in range(2):
                        wsl = pw.tile([P, DS, NCH], BF16, tag="w")
                        nc.sync.dma_start(
                            wsl[:],
                            WvT_d[l].rearrange("(po pi) o -> pi po o", pi=P)[:, :, bass.ts(half, NCH)])
                        bvs = pbv.tile([1, NCH], BF16, tag="bvs")
                        nc.sync.dma_start(bvs[:], bv_d[:, l * D + half * NCH:l * D + (half + 1) * NCH])
                        for t_ in range(TS):
                            ps_t = pps.tile([P, NCH], F32, tag="a")
                            for k in range(DS):
                                nc.tensor.matmul(ps_t[:], hT[:, k, bass.ts(t_, P)],
                                                 wsl[:, k, :],
                                                 start=(k == 0), stop=False)
                            nc.tensor.matmul(ps_t[:], ones_r[:], bvs[:],
                                             start=False, stop=True)
                            nc.vector.tensor_copy(
                                Vg[:, t_, 8 * half:8 * half + 8, 0:HD],
                                ps_t[:].rearrange("p (h d) -> p h d", d=HD))

                    # ---- attention, per head ----
                    for h in range(H):
                        pbase = (h % 2) * HD
                        sub = h // 2
                        for c in range(2):
                            PT = ppt.tile([P, TS, NCH], BF16, tag="pt")
                            ntk = 4 * c + 4
                            for tk in range(ntk):
                                ls = max(0, tk * P - c * NCH)
                                w_ = NCH - ls
                                sT = pps.tile([P, NCH], F32, tag="a")
                                nc.tensor.matmul(
                                    sT[:, :w_],
                                    qkT[pbase:pbase + HD, DS + sub, bass.ts(tk, P)],
                                    qkT[pbase:pbase + HD, sub, c * NCH + ls:(c + 1) * NCH],
                                    start=True, stop=True)
                                nc.scalar.activation(PT[:, tk, ls:], sT[:, :w_], AF.Exp,
                                                     scale=float(SCALE))
                                if tk >= 4 * c:
                                    nc.vector.tensor_mul(PT[:, tk, ls:ls + P],
                                                         PT[:, tk, ls:ls + P], mask[:])
                            po_t = ppo.tile([HD + 1, NCH], F32, tag="o")
                            for tk in range(ntk):
                                ls = max(0, tk * P - c * NCH)
                                nc.tensor.matmul(po_t[:, ls:], Vg[:, tk, h, :],
                                                 PT[:, tk, ls:],
                                                 start=(tk == 0), stop=(tk == ntk - 1))
                            dn = pr.tile([1, NCH], BF16, tag="rb")
                            with nc.allow_low_precision(reason="attn denom bf16"):
                                nc.vector.reciprocal(dn[:], po_t[HD:HD + 1, :])
                            bc = ppb.tile([P, NCH], F32, tag="b")
                            nc.tensor.matmul(bc[:HD, :], ones_rf[:, :HD], dn[:],
                                             start=True, stop=True)
                            yu = px.tile([HD, NCH], BF16, tag="yu")
                            nc.vector.tensor_copy(yu[:], po_t[:HD, :])
                            nc.vector.tensor_mul(yT[pbase:pbase + HD, sub, bass.ts(c, NCH)],
                                                 yu[:], bc[:HD, :])

                    # ---- attention out projection + residual ----
                    for half in range(2):
                        wsl = pw.tile([P, DS, NCH], BF16, tag="w")
                        nc.sync.dma_start(
                            wsl[:],
                            WoT_d[l].rearrange("(po pi) o -> pi po o", pi=P)[:, :, bass.ts(half, NCH)])
                        for m in range(4):
                            mo = half * 4 + m
                            ps0 = pps.tile([P, NCH], F32, tag="a")
                            ps1 = pps.tile([P, NCH], F32, tag="a")
                            for k in range(DS):
                                nc.tensor.matmul(ps0[:], wsl[:, k, bass.ts(m, P)],
                                                 yT[:, k, bass.ts(0, NCH)],
                                                 start=(k == 0), stop=(k == DS - 1))
                                nc.tensor.matmul(ps1[:], wsl[:, k, bass.ts(m, P)],
                                                 yT[:, k, bass.ts(1, NCH)],
                                                 start=(k == 0), stop=(k == DS - 1))
                            for c, psx in ((0, ps0), (1, ps1)):
                                tch = bass.ts(c, NCH)
                                nc.vector.scalar_tensor_tensor(
                                    xT[:, mo, tch], psx[:],
                                    sm[:, base + SM_BO + mo:base + SM_BO + mo + 1],
                                    xT[:, mo, tch], op0=ALU.add, op1=ALU.add)

                    # ---- MLP ----
                    _layernorm(nc, pps, ppb, px, pr, xT, hT, sm,
                               base + SM_L2W, base + SM_L2B, ones_cb, ones_rf, eps_t)
                    for quarter in range(4):
                        wsl = pw.tile([P, DS, NCH], BF16, tag="w")
                        nc.sync.dma_start(
                            wsl[:],
                            W1T_d[l].rearrange("(po pi) o -> pi po o", pi=P)[:, :, bass.ts(quarter, NCH)])
                        for m in range(4):
                            mo = quarter * 4 + m
                            ps0 = pps.tile([P, NCH], F32, tag="a")
                            ps1 = pps.tile([P, NCH], F32, tag="a")
                            for k in range(DS):
                                nc.tensor.matmul(ps0[:], wsl[:, k, bass.ts(m, P)],
                                                 hT[:, k, bass.ts(0, NCH)],
                                                 start=(k == 0), stop=(k == DS - 1))
                                nc.tensor.matmul(ps1[:], wsl[:, k, bass.ts(m, P)],
                                                 hT[:, k, bass.ts(1, NCH)],
                                                 start=(k == 0), stop=(k == DS - 1))
                            for c, psx in ((0, ps0), (1, ps1)):
                                nc.scalar.activation(
                                    qkT[:, mo, bass.ts(c, NCH)], psx[:], AF.Gelu,
                                    bias=sm[:, base + SM_B1 + mo:base + SM_B1 + mo + 1])

                    for half in range(2):
                        w2t = pw2.tile([P, D2S, NCH], BF16, tag="w2")
                        nc.sync.dma_start(
                            w2t[:],
                            W2T_d[l].rearrange("(po pi) o -> pi po o", pi=P)[:, :, bass.ts(half, NCH)])
                        for m in range(4):
                            mo = half * 4 + m
                            ps0 = pps.tile([P, NCH], F32, tag="a")
                            ps1 = pps.tile([P, NCH], F32, tag="a")
                            for k in range(D2S):
                                nc.tensor.matmul(ps0[:], w2t[:, k, bass.ts(m, P)],
                                                 qkT[:, k, bass.ts(0, NCH)],
                                                 start=(k == 0), stop=(k == D2S - 1))
                                nc.tensor.matmul(ps1[:], w2t[:, k, bass.ts(m, P)],
                                                 qkT[:, k, bass.ts(1, NCH)],
                                                 start=(k == 0), stop=(k == D2S - 1))
                            for c, psx in ((0, ps0), (1, ps1)):
                                tch = bass.ts(c, NCH)
                                nc.vector.scalar_tensor_tensor(
                                    xT[:, mo, tch], psx[:],
                                    sm[:, base + SM_B2 + mo:base + SM_B2 + mo + 1],
                                    xT[:, mo, tch], op0=ALU.add, op1=ALU.add)

                # ---- final LN + vocab head ----
                _layernorm(nc, pps, ppb, px, pr, xT, hT, sm,
                           SM_LNFW, SM_LNFB, ones_cb, ones_rf, eps_t)
                hw_r = hT_w_d[:, :].rearrange("(po pi) v -> pi po v", pi=P)
                for vp in range(VS // 2):
                    ws0 = pw.tile([P, DS, NCH], BF16, tag="w")
                    nc.sync.dma_start(ws0[:], hw_r[:, :, bass.ts(2 * vp, NCH)])
                    ws1 = pw.tile([P, DS, NCH], BF16, tag="w")
                    nc.sync.dma_start(ws1[:], hw_r[:, :, bass.ts(2 * vp + 1, NCH)])
                    for t_ in range(TS):
                        ps0 = pps.tile([P, NCH], F32, tag="a")
                        ps1 = pps.tile([P, NCH], F32, tag="a")
                        for k in range(DS):
                            nc.tensor.matmul(ps0[:], hT[:, k, bass.ts(t_, P)],
                                             ws0[:, k, :],
                                             start=(k == 0), stop=(k == DS - 1))
                            nc.tensor.matmul(ps1[:], hT[:, k, bass.ts(t_, P)],
                                             ws1[:, k, :],
                                             start=(k == 0), stop=(k == DS - 1))
                        for j, psx in ((0, ps0), (1, ps1)):
                            ot = px.tile([P, NCH], F32, tag="ot")
                            nc.vector.tensor_copy(ot[:], psx[:])
                            nc.sync.dma_start(out_r[:, t_, bass.ts(2 * vp + j, NCH)], ot[:])

    nc.compile()
    return nc


_NC = {}


def _get_nc(repeat=1):
    if repeat not in _NC:
        _NC[repeat] = _build(repeat)
    return _NC[repeat]


def _pack_cols(vec, ncols):
    """[ncols*128] -> [128, ncols] with column j = vec[j*128:(j+1)*128]."""
    return np.ascontiguousarray(vec.reshape(ncols, P).T)


def kernel(idx, timesteps, tok_emb_w, pos_emb, global_pos_emb,
           ln1_w, ln1_b, Wq, bq, Wk, bk, Wv, bv, Wo, bo,
           ln2_w, ln2_b, W1, b1, W2, b2, lnf_w, lnf_b, head_w):
    global LAST_RESULTS
    f = lambda a: np.asarray(a, dtype=np.float32)
    idx = np.asarray(idx, dtype=np.int64)
    tsteps = np.asarray(timesteps, dtype=np.int64)
    tok_emb_w, pos_emb, global_pos_emb = f(tok_emb_w), f(pos_emb), f(global_pos_emb)

    # embedding on host (tiny compute, avoids on-device gather)
    x0 = tok_emb_w[idx] + global_pos_emb[0][tsteps[:, 0]][:, None, :] + pos_emb[:, :T]
    x0 = np.ascontiguousarray(x0.astype(np.float32))

    bf = lambda a: np.ascontiguousarray(np.asarray(a, np.float32)).astype(ml_dtypes.bfloat16)
    # Fold LN gamma into the consuming projections (device computes plain
    # (x-mu)/sigma): WqT' = g1[:,None] * Wq.T; beta folds into the biases:
    # bq' = Wq @ b_ln1 + bq. Same for k, v, W1 (ln2) and the head (lnf).
    Wqf, Wkf, Wvf, W1f = f(Wq), f(Wk), f(Wv), f(W1)
    b1l, b2l = f(ln1_b), f(ln2_b)
    bqf = np.einsum('lod,ld->lo', Wqf, b1l) + f(bq)
    bkf = np.einsum('lod,ld->lo', Wkf, b1l) + f(bk)
    bvf = np.einsum('lod,ld->lo', Wvf, b1l) + f(bv)
    b1f = np.einsum('lod,ld->lo', W1f, b2l) + f(b1)
    head_bias = f(head_w) @ f(lnf_b)           # [V]; added host-side post-gather
    shared = {
        "WqT": bf(f(ln1_w)[:, :, None] * Wqf.transpose(0, 2, 1)),
        "WkT": bf(f(ln1_w)[:, :, None] * Wkf.transpose(0, 2, 1)),
        "WvT": bf(f(ln1_w)[:, :, None] * Wvf.transpose(0, 2, 1)),
        "WoT": bf(f(Wo).transpose(0, 2, 1)),
        "W1T": bf(f(ln2_w)[:, :, None] * W1f.transpose(0, 2, 1)),
        "W2T": bf(f(W2).transpose(0, 2, 1)),
        "headT": bf(f(lnf_w)[:, None] * f(head_w).T),
        "bvB": bf(bvf.reshape(1, L * D)),
    }
    smalls = np.zeros((P, SM_COLS), np.float32)
    for l in range(L):
        b = l * SM_PER_LAYER
        smalls[:, b + SM_BQ:b + SM_BQ + 8] = _pack_cols(bqf[l], DS)
        smalls[:, b + SM_BK:b + SM_BK + 8] = _pack_cols(bkf[l], DS)
        smalls[:, b + SM_BO:b + SM_BO + 8] = _pack_cols(f(bo)[l], DS)
        smalls[:, b + SM_B2:b + SM_B2 + 8] = _pack_cols(f(b2)[l], DS)
        smalls[:, b + SM_B1:b + SM_B1 + 16] = _pack_cols(b1f[l], D2S)
        smalls[:, b + SM_L1W:b + SM_L1W + 8] = _pack_cols(f(ln1_w)[l], DS)
        smalls[:, b + SM_L1B:b + SM_L1B + 8] = _pack_cols(f(ln1_b)[l], DS)
        smalls[:, b + SM_L2W:b + SM_L2W + 8] = _pack_cols(f(ln2_w)[l], DS)
        smalls[:, b + SM_L2B:b + SM_L2B + 8] = _pack_cols(f(ln2_b)[l], DS)
    smalls[:, SM_LNFW:SM_LNFW + 8] = _pack_cols(f(lnf_w), DS)
    smalls[:, SM_LNFB:SM_LNFB + 8] = _pack_cols(f(lnf_b), DS)
    shared["smalls"] = smalls

    m01 = (np.arange(P)[:, None] <= np.arange(P)[None, :])
    shared["mask01"] = m01.astype(ml_dtypes.bfloat16)

    in_maps = []
    for b_ in range(B):
        m = dict(shared)
        m["x0T"] = np.ascontiguousarray(x0[b_].T)
        in_maps.append(m)

    global LAST_IN_MAPS
    LAST_IN_MAPS = in_maps
    nc = _get_nc()
    res = run_bass_kernel_spmd(nc, in_maps, core_ids=list(range(B)), trace=TRACE)
    LAST_RESULTS = res
    out = np.stack([np.asarray(res.results[c]["logits"], np.float32) for c in range(B)])
    if np.any(head_bias):
        out = out + head_bias[None, None, :]
    return out


# ---------------------------------------------------------------------------
# Timing helpers (test-only): replicate run_bass_via_pjrt's sharded jit with
# device-resident inputs so repeated calls measure (dispatch + NEFF exec).
# ---------------------------------------------------------------------------
LAST_IN_MAPS = None


def _sharded_exec(nc, in_maps):
    import jax
    from jax.experimental.shard_map import shard_map
    from jax.sharding import Mesh, PartitionSpec
    from concourse import bass2jax

    bass2jax.install_neuronx_cc_hook()
    n_cores = len(in_maps)
    partition_name = nc.partition_id_tensor.name if nc.partition_id_tensor else None
    in_names, out_names, out_avals, zero_outs = [], [], [], []
    for alloc in nc.m.functions[0].allocations:
        if not isinstance(alloc, mybir.MemoryLocationSet):
            continue
        name = alloc.memorylocations[0].name
        if alloc.kind == "ExternalInput":
            if name != partition_name:
                in_names.append(name)
        elif alloc.kind == "ExternalOutput":
            shape = tuple(alloc.tensor_shape)
            dtype = mybir.dt.np(alloc.dtype)
            out_names.append(name)
            out_avals.append(jax.core.ShapedArray(shape, dtype))
            zero_outs.append(np.zeros(shape, dtype))
    n_params = len(in_names)
    n_outs = len(out_avals)
    all_in_names = list(in_names) + list(out_names)
    if partition_name is not None:
        all_in_names.append(partition_name)
    donate = tuple(range(n_params, n_params + n_outs))

    def _body(*args):
        operands = list(args)
        if partition_name is not None:
            operands.append(bass2jax.partition_id_tensor())
        outs = bass2jax._bass_exec_p.bind(
            *operands,
            out_avals=tuple(out_avals),
            in_names=tuple(all_in_names),
            out_names=tuple(out_names),
            lowering_input_output_aliases=(),
            sim_require_finite=True,
            sim_require_nnan=True,
            nc=nc,
        )
        return tuple(outs)

    devices = jax.devices()[:n_cores]
    mesh = Mesh(np.asarray(devices), ("core",))
    sharded = jax.jit(
        shard_map(_body, mesh=mesh,
                  in_specs=(PartitionSpec("core"),) * (n_params + n_outs),
                  out_specs=(PartitionSpec("core"),) * n_outs,
                  check_rep=False),
        donate_argnums=donate, keep_unused=True)

    concat_in = [np.concatenate([np.asarray(m[name]) for m in in_maps], axis=0)
                 for name in in_names]
    concat_zeros = [np.zeros((n_cores * z.shape[0], *z.shape[1:]), z.dtype)
                    for z in zero_outs]
    from jax.sharding import NamedSharding
    sh = NamedSharding(mesh, PartitionSpec("core"))
    dev_in = [jax.device_put(a, sh) for a in concat_in]
    return sharded, dev_in, concat_zeros, sh


def _time_exec(nc, in_maps, iters):
    import time as _time
    import jax
    sharded, dev_in, concat_zeros, sh = _sharded_exec(nc, in_maps)
    times = []
    for _ in range(iters):
        zs = [jax.device_put(z, sh) for z in concat_zeros]
        jax.block_until_ready(zs)
        jax.block_until_ready(dev_in)
        t0 = _time.perf_counter()
        out = sharded(*dev_in, *zs)
        jax.block_until_ready(out)
        times.append(_time.perf_counter() - t0)
    return times


def timed_run(iters=5):
    assert LAST_IN_MAPS is not None, "call kernel() first"
    return _time_exec(_get_nc(), LAST_IN_MAPS, iters)


_BASELINE_NC = None


def _build_baseline():
    nc = bacc.Bacc("TRN2", target_bir_lowering=False)
    a_d = nc.dram_tensor("a", [P, P], F32, kind="ExternalInput")
    o_d = nc.dram_tensor("o", [P, P], F32, kind="ExternalOutput")
    with tile.TileContext(nc) as tc:
        with tc.tile_pool(name="s", bufs=1) as s:
            t = s.tile([P, P], F32)
            nc.sync.dma_start(t[:], a_d[:, :])
            nc.sync.dma_start(o_d[:, :], t[:])
    nc.compile()
    return nc


def timed_baseline(iters=5):
    global _BASELINE_NC
    if _BASELINE_NC is None:
        _BASELINE_NC = _build_baseline()
    maps = [{"a": np.zeros((P, P), np.float32)} for _ in range(B)]
    return _time_exec(_BASELINE_NC, maps, iters)


def timed_slope(ns=(1, 4, 12), zsets=12):
    """Async-dispatch n calls back-to-back; slope of total-time vs n ~ exec."""
    import time as _time
    import jax
    assert LAST_IN_MAPS is not None
    sharded, dev_in, concat_zeros, sh = _sharded_exec(_get_nc(), LAST_IN_MAPS)
    all_zs = [[jax.device_put(z, sh) for z in concat_zeros] for _ in range(zsets)]
    jax.block_until_ready(all_zs)
    jax.block_until_ready(dev_in)
    # warm
    out = sharded(*dev_in, *all_zs[0])
    jax.block_until_ready(out)
    res = {}
    for n in ns:
        zs_fresh = [[jax.device_put(z, sh) for z in concat_zeros] for _ in range(n)]
        jax.block_until_ready(zs_fresh)
        t0 = _time.perf_counter()
        outs = [sharded(*dev_in, *zs_fresh[i]) for i in range(n)]
        jax.block_until_ready(outs)
        res[n] = _time.perf_counter() - t0
    return res


def timed_slope_nc(nc, in_maps, ns=(4, 20)):
    import time as _time
    import jax
    sharded, dev_in, concat_zeros, sh = _sharded_exec(nc, in_maps)
    out = sharded(*dev_in, *[jax.device_put(z, sh) for z in concat_zeros])
    jax.block_until_ready(out)
    res = {}
    for n in ns:
        zs_fresh = [[jax.device_put(z, sh) for z in concat_zeros] for _ in range(n)]
        jax.block_until_ready(zs_fresh)
        jax.block_until_ready(dev_in)
        t0 = _time.perf_counter()
        outs = [sharded(*dev_in, *zs_fresh[i]) for i in range(n)]
        jax.block_until_ready(outs)
        res[n] = _time.perf_counter() - t0
    return res


def timed_repeat(r=5, iters=6):
    """exec_ns ~= (min_time(R=r NEFF) - min_time(R=1 NEFF)) / (r-1)."""
    assert LAST_IN_MAPS is not None
    t1 = min(_time_exec(_get_nc(1), LAST_IN_MAPS, iters))
    tr = min(_time_exec(_get_nc(r), LAST_IN_MAPS, iters))
    return (tr - t1) / (r - 1), t1, tr

